# revision 9
# baseline (speedup 1.0000x reference)
"""FAISS-anchor kernel layer on 8 Trainium2 NeuronCores (Bass/Tile).

Problem (per full input):
    x [8,8192,3], Key [1024,3], init_mat/Value [1024,256],
    w1 [3,1024], b1 [1024], w2 [1024,256], b2 [256]
    idx = argmin_a ||x - Key_a||^2           (exact 1-NN, first-tie)
    out = gelu((x - Key[idx]) @ w1 + b1) @ w2 + b2 + (init_mat + Value)[idx]

Sharding: pure data-parallel — core c takes batch element c (8192 tokens).
All tables (Key-derived features, V-table, MLP weights) are replicated.

Device pipeline, software-pipelined in 16 half-chunk steps of 4 token
tiles (512 tokens) so the PE instruction stream stays gapless (TRN2 PE
DVFS only reaches 2.4 GHz after ~3us of continuous execution):
  step k    PE:   s = -||x-k||^2 for 1024 anchors (2 row-group-packed
                  f32r matmuls per tile, interleaved with out-GEMMs of
                  step k-4)
            DVE:  max8 -> top-8 of s; max_index -> argmin anchor idx
            DMA:  indirect gather of fused row [V+init+b2 | 2*Key] (f32)
  step k+1  PE:   transpose gathered 2*Key rows -> [4, tok] (PSUM)
            GPS:  rlts = xh[0:4] - ket  (= [2x - 2Key[idx]; junk]^T)
  step k+2  PE:   h^T = (0.5*w1)^T @ rlts (K=4)
            ACT:  ht = gelu(h^T + b1) -> bf16
  step k+4  PE:   o = ht^T @ w2(bf16) accumulated over 8 K-chunks, then
                  += V-row via identity matmul; DMA PSUM -> DRAM.

Host: packs layouts, runs 8 cores via run_bass_kernel_spmd, re-assembles,
and re-resolves near-tie tokens (top-2 gap below tau) with exact fp32
reference arithmetic so fp32r matmul rounding cannot flip the argmin.
"""

import numpy as np

B, N, A, D_IN, D_OUT = 8, 8192, 1024, 3, 256
H = 4 * D_OUT
P = 128
NT = N // P            # 64 token tiles per core
TPS = 4                # token tiles per pipeline step
NSTEP = NT // TPS      # 16 steps
SC = TPS * P           # 512 tokens per step
VT_W = 272             # gather-table row width (256 V + 3 key + pad), 1088B
HC = H // P            # 8 H-chunks
TR_LAG = 2             # transpose+subtract phase lag (steps)
H_LAG = 3              # h-MLP phase lag (steps)
OG_LAG = 5             # out-GEMM phase lag (steps)
N_CORES = 8

_PROGRAM = None  # (nc, input_names)


def _build_program():
    import concourse.bass as bass
    import concourse.mybir as mybir
    import concourse.tile as tile
    from concourse import bacc

    f32 = mybir.dt.float32
    f32r = mybir.dt.float32r
    bf16 = mybir.dt.bfloat16
    u32 = mybir.dt.uint32

    # Bacc (not raw Bass): its compile() splits multi-sem waits and moves
    # matmul waits onto ldweights — TRN2 allows at most 1 wait per instr.
    nc = bacc.Bacc("TRN2", target_bir_lowering=False, debug=False)

    # DRAM I/O (xh/kh hold feature rows 0-4 duplicated at 5-9 so they can
    # be DMA'd to SBUF partitions 0-4 and 32-36 for row-group packing).
    xh_d = nc.dram_tensor("xh", [10, N], f32r, kind="ExternalInput").ap()
    kh_d = nc.dram_tensor("kh", [10, A], f32r, kind="ExternalInput").ap()
    w1h_d = nc.dram_tensor("w1h", [4, H], f32r, kind="ExternalInput").ap()
    b1p_d = nc.dram_tensor("b1p", [P, HC], f32, kind="ExternalInput").ap()
    w2p_d = nc.dram_tensor("w2p", [P, HC * D_OUT], bf16, kind="ExternalInput").ap()
    vt_d = nc.dram_tensor("vt", [A, VT_W], f32, kind="ExternalInput").ap()
    id_d = nc.dram_tensor("ident", [P, P], f32r, kind="ExternalInput").ap()

    out_d = nc.dram_tensor("outp", [N, D_OUT], f32, kind="ExternalOutput").ap()
    m8_d = nc.dram_tensor("m8o", [P, NT * 8], f32, kind="ExternalOutput").ap()
    idx_d = nc.dram_tensor("idxo", [P, NT * 8], u32, kind="ExternalOutput").ap()

    with tile.TileContext(nc) as tc:
        with (
            tc.tile_pool(name="const", bufs=1) as cpool,
            tc.tile_pool(name="xh", bufs=6) as xhpool,
            tc.tile_pool(name="vg", bufs=28) as vgpool,
            tc.tile_pool(name="rlts", bufs=4) as rltspool,
            tc.tile_pool(name="ob", bufs=4) as obpool,
            tc.tile_pool(name="ht", bufs=40) as htpool,
            tc.tile_pool(name="m8", bufs=1) as m8pool,
            tc.tile_pool(name="idx", bufs=6) as idxpool,
            tc.tile_pool(name="ps", bufs=2, space="PSUM") as pspool,
        ):
            # Resident constants. kh first (needed by the very first s-mm).
            kh_t = cpool.tile([P, A], f32r)
            nc.sync.dma_start(out=kh_t[0:5, :], in_=kh_d[0:5, :])
            nc.sync.dma_start(out=kh_t[32:37, :], in_=kh_d[5:10, :])
            w1h_t = cpool.tile([4, H], f32r)
            nc.sync.dma_start(out=w1h_t[:], in_=w1h_d[:])
            b1p_t = cpool.tile([P, HC], f32)
            nc.sync.dma_start(out=b1p_t[:], in_=b1p_d[:])
            id_t = cpool.tile([P, P], f32r)
            nc.sync.dma_start(out=id_t[:], in_=id_d[:])
            w2p_t = cpool.tile([P, HC * D_OUT], bf16)
            nc.sync.dma_start(out=w2p_t[:], in_=w2p_d[:])
            m8_t = m8pool.tile([P, NT * 8], f32)

            xh_ts = {}

            def fetch_xh(k):
                xh_t = xhpool.tile([P, SC], f32r, tag="xh")
                nc.sync.dma_start(out=xh_t[0:5, :], in_=xh_d[0:5, k * SC : (k + 1) * SC])
                nc.sync.dma_start(out=xh_t[32:37, :], in_=xh_d[5:10, k * SC : (k + 1) * SC])
                xh_ts[k] = xh_t

            fetch_xh(0)
            fetch_xh(1)

            vgs = {}      # k -> [vg tile per tile-in-step]
            idxs = {}     # k -> idx tile [P, TPS, 8]
            rlts_k = {}   # k -> rlts tile [4, SC]
            hts = {}      # k -> [ht tile per hc]

            def emit_s_pair(k, j):
                """Score matmuls + DVE scan for tile j of step k."""
                t = k * TPS + j
                s_ps = pspool.tile([P, A], f32, tag="s")
                for g in range(2):
                    nc.tensor.matmul(
                        out=s_ps[:, g * 512 : (g + 1) * 512],
                        lhsT=xh_ts[k][32 * g : 32 * g + 5, j * P : (j + 1) * P],
                        rhs=kh_t[32 * g : 32 * g + 5, g * 512 : (g + 1) * 512],
                        start=True,
                        stop=True,
                        tile_position=(32 * g, 0),
                    )
                m8 = m8_t[:, t * 8 : (t + 1) * 8]
                nc.vector.max(m8, s_ps[:])
                nc.vector.max_index(idxs[k][:, j, :], m8, s_ps[:])

            def emit_gather(k, j):
                # NB: one offset per partition ([P,1]) per call into an
                # offset-0 [P, W] dest tile — both multi-index offsets and
                # non-zero dest offsets are mishandled by the real SWDGE
                # (CoreSim accepts them but hardware does not).
                vg_j = vgpool.tile([P, VT_W], f32r, tag="vg")
                nc.gpsimd.indirect_dma_start(
                    out=vg_j[:],
                    out_offset=None,
                    in_=vt_d[:],
                    in_offset=bass.IndirectOffsetOnAxis(ap=idxs[k][:, j, 0:1], axis=0),
                )
                vgs[k].append(vg_j)

            def emit_og(k, j):
                """Out-GEMM for tile j of step k: 8 bf16 K-chunks + V row."""
                t = k * TPS + j
                o_ps = pspool.tile([P, D_OUT], f32, tag="o")
                for hc in range(HC):
                    nc.tensor.matmul(
                        out=o_ps[:],
                        lhsT=hts[k][hc][:, j * P : (j + 1) * P],
                        rhs=w2p_t[:, hc * D_OUT : (hc + 1) * D_OUT],
                        start=(hc == 0),
                        stop=False,
                    )
                nc.tensor.matmul(
                    out=o_ps[:],
                    lhsT=id_t[:],
                    rhs=vgs[k][j][:, 0:D_OUT],
                    start=False,
                    stop=True,
                )
                ob = obpool.tile([P, D_OUT], f32, tag="ob", name="ob")
                nc.scalar.activation(
                    out=ob[:], in_=o_ps[:],
                    func=mybir.ActivationFunctionType.Copy,
                )
                nc.sync.dma_start(out=out_d[t * P : (t + 1) * P, :], in_=ob[:])

            def emit_h_one(k, hc):
                """One h-chunk matmul + gelu for step k."""
                if True:
                    h_ps = pspool.tile([P, SC], f32, tag="h", name="hps")
                    nc.tensor.matmul(
                        out=h_ps[:],
                        lhsT=w1h_t[:, hc * P : (hc + 1) * P],
                        rhs=rlts_k[k][:],
                        start=True,
                        stop=True,
                    )
                    ht = htpool.tile([P, SC], bf16, tag="ht", name="ht")
                    nc.scalar.activation(
                        out=ht[:],
                        in_=h_ps[:],
                        func=mybir.ActivationFunctionType.Gelu,
                        bias=b1p_t[:, hc : hc + 1],
                    )
                    hts[k].append(ht)

            for k in range(NSTEP + OG_LAG):
                a = k < NSTEP                          # score/scan/gather
                trp = TR_LAG <= k < NSTEP + TR_LAG     # transpose+subtract
                hp = H_LAG <= k < NSTEP + H_LAG        # h-MLP
                og = OG_LAG <= k                       # out-GEMM

                if a:
                    if k + 2 < NSTEP:
                        fetch_xh(k + 2)
                    idxs[k] = idxpool.tile([P, TPS, 8], u32, tag="idx", name="idxt")
                    vgs[k] = []
                if hp:
                    hts[k - H_LAG] = []

                # PE order per slot j: s-pair(k) | out-GEMM group(k-5) |
                # 2 h-chunks(k-3), with transposes(k-2)+subtract after
                # slot 1.  The deep lags give every cross-engine dep >1
                # step of slack; the slot interleave keeps each PSUM ring
                # (s bufs=2, h bufs=2, o bufs=2) comfortably ahead of its
                # consumers so the PE stream never stalls (any bubble
                # drops the PE clock from 2.4GHz to 1.2GHz for ~3us).
                for j in range(TPS):
                    if a:
                        emit_s_pair(k, j)
                    if hp:
                        emit_h_one(k - H_LAG, 2 * j)
                    if og:
                        emit_og(k - OG_LAG, j)
                    if hp:
                        emit_h_one(k - H_LAG, 2 * j + 1)
                    if j == 1 and trp:
                        kk = k - TR_LAG
                        rlt_ps = pspool.tile([4, SC], f32r, tag="s", name="rltps")
                        for jj in range(TPS):
                            nc.tensor.transpose(
                                out=rlt_ps[:, jj * P : (jj + 1) * P],
                                in_=vgs[kk][jj][:, D_OUT : D_OUT + 4],
                                identity=id_t[:],
                            )
                        rl = rltspool.tile([4, SC], f32r, tag="rlts", name="rl")
                        nc.vector.tensor_tensor(
                            out=rl[:],
                            in0=xh_ts[kk][0:4, :],
                            in1=rlt_ps[:],
                            op=mybir.AluOpType.subtract,
                        )
                        rlts_k[kk] = rl

                if a:
                    # gathers follow their scans on GpSimd
                    for j in range(TPS):
                        emit_gather(k, j)
                    nc.sync.dma_start(
                        out=idx_d[:, k * TPS * 8 : (k + 1) * TPS * 8],
                        in_=idxs[k][:],
                    )

            nc.sync.dma_start(out=m8_d[:], in_=m8_t[:])

    nc.compile()
    names = ["xh", "kh", "w1h", "b1p", "w2p", "vt", "ident"]
    return nc, names


def _get_program():
    global _PROGRAM
    if _PROGRAM is None:
        _PROGRAM = _build_program()
    return _PROGRAM


def _host_pack(x, Key, init_mat, Value, w1, b1, w2, b2):
    """Build per-core input dicts (host-side layout packing)."""
    import ml_dtypes

    f = np.float32
    Key = np.asarray(Key, f)
    x = np.asarray(x, f)
    k2 = np.sum(Key * Key, axis=1)  # [A]

    # khat rows: [k0,k1,k2,1,|k|^2]; s = 2x.k - |x|^2 - |k|^2 = -d2
    kf = np.concatenate([Key, np.ones((A, 1), f), k2[:, None]], axis=1)  # [A,5]
    kh = np.concatenate([kf.T, kf.T], axis=0)  # [10, A]

    w1h = np.zeros((4, H), f)
    w1h[:3, :] = 0.5 * np.asarray(w1, f)
    b1p = np.asarray(b1, f).reshape(HC, P).T.copy()  # [128, 8]
    w2p = (
        np.asarray(w2, f)
        .reshape(HC, P, D_OUT)
        .transpose(1, 0, 2)
        .reshape(P, HC * D_OUT)
        .astype(ml_dtypes.bfloat16)
    )
    vt = np.zeros((A, VT_W), f)
    vt[:, :D_OUT] = np.asarray(init_mat, f) + np.asarray(Value, f) + np.asarray(b2, f)
    vt[:, D_OUT : D_OUT + 3] = 2.0 * Key
    ident = np.eye(P, dtype=f)

    in_maps = []
    for c in range(N_CORES):
        xc = x[c]  # [N, 3]
        x2sq = np.sum(xc * xc, axis=1)  # [N]
        # xhat features [N, 5]: [2x, -|x|^2, -1]
        xf = np.concatenate(
            [2.0 * xc, -x2sq[:, None], -np.ones((N, 1), f)], axis=1
        ).astype(f)
        # [5, N] with tile t at cols t*128.. (tile-major token order)
        xf_t = xf.reshape(NT, P, 5).transpose(2, 0, 1).reshape(5, N)
        xh = np.concatenate([xf_t, xf_t], axis=0).copy()  # [10, N]

        in_maps.append(
            {
                "xh": xh,
                "kh": kh,
                "w1h": w1h,
                "b1p": b1p,
                "w2p": w2p,
                "vt": vt,
                "ident": ident,
            }
        )
    return in_maps


def _erf(z):
    # Abramowitz-Stegun is not enough; use the exact erf from scipy if
    # present, else jax (available wherever the bass stack runs).
    try:
        from scipy.special import erf

        return erf(z)
    except ImportError:
        import jax

        with jax.default_device(jax.devices("cpu")[0]):
            return np.asarray(jax.scipy.special.erf(np.asarray(z, np.float32)))


def _refine(out, m8o, idxo, x, Key, init_mat, Value, w1, b1, w2, b2, tau=0.03):
    """Re-resolve tokens whose top-2 score gap is within tau (near-ties):
    recompute their argmin + output row in exact fp32 reference arithmetic."""
    f = np.float32
    Key = np.asarray(Key, f)
    V = np.asarray(init_mat, f) + np.asarray(Value, f)
    k2 = np.sum(Key * Key, axis=1)
    n_fixed = 0
    for c in range(out.shape[0]):
        m8 = m8o[c]  # [128, NT*8]
        m0 = m8[:, 0::8]  # [128, NT]
        m1 = m8[:, 1::8]
        gap = m0 - m1  # s-space gap == d2 second - d2 min
        dev_idx = idxo[c][:, 0::8].astype(np.int64)  # [128, NT]
        scale = 1.0 + np.abs(m0)
        flag = gap < tau * scale  # [128, NT]
        ps, ts = np.nonzero(flag)
        if ps.size == 0:
            continue
        toks = ts * P + ps
        xc = np.asarray(x[c], f)[toks]  # [F, 3]
        d2 = -2.0 * (xc @ Key.T) + k2[None, :]  # reference formula, fp32
        amin = np.argmin(d2, axis=1)
        mism = amin != dev_idx[ps, ts]
        if not np.any(mism):
            continue
        toks = toks[mism]
        amin = amin[mism]
        xe = np.asarray(x[c], f)[toks]
        rl = xe - Key[amin]
        pre = (rl @ np.asarray(w1, f) + np.asarray(b1, f)).astype(f)
        h = (0.5 * pre * (1.0 + _erf(pre / np.sqrt(f(2.0))))).astype(f)
        row = (h @ np.asarray(w2, f) + np.asarray(b2, f) + V[amin]).astype(f)
        out[c, toks, :] = row
        n_fixed += toks.size
    return n_fixed


def kernel(**inputs):
    from concourse.bass_utils import run_bass_kernel_spmd

    nc, names = _get_program()
    in_maps = _host_pack(**inputs)
    res = run_bass_kernel_spmd(nc, in_maps, core_ids=list(range(N_CORES)))

    out = np.zeros((B, N, D_OUT), np.float32)
    m8o = np.zeros((B, P, NT * 8), np.float32)
    idxo = np.zeros((B, P, NT * 8), np.uint32)
    for c in range(N_CORES):
        r = res.results[c]
        out[c] = r["outp"]
        m8o[c] = r["m8o"]
        idxo[c] = r["idxo"]

    _refine(out, m8o, idxo, **inputs)
    return out


if __name__ == "__main__":
    # smoke: build only
    _get_program()
    print("program built")


# revision 10
# speedup vs baseline: 1.0664x; 1.0664x over previous
"""FAISS-anchor kernel layer on 8 Trainium2 NeuronCores (Bass/Tile).

Problem (per full input):
    x [8,8192,3], Key [1024,3], init_mat/Value [1024,256],
    w1 [3,1024], b1 [1024], w2 [1024,256], b2 [256]
    idx = argmin_a ||x - Key_a||^2           (exact 1-NN, first-tie)
    out = gelu((x - Key[idx]) @ w1 + b1) @ w2 + b2 + (init_mat + Value)[idx]

Sharding: pure data-parallel — core c takes batch element c (8192 tokens).
All tables (Key-derived features, V-table, MLP weights) are replicated.

Device pipeline, software-pipelined in 16 half-chunk steps of 4 token
tiles (512 tokens) so the PE instruction stream stays gapless (TRN2 PE
DVFS only reaches 2.4 GHz after ~3us of continuous execution):
  step k    PE:   s = -||x-k||^2 for 1024 anchors (2 row-group-packed
                  f32r matmuls per tile, interleaved with out-GEMMs of
                  step k-4)
            DVE:  max8 -> top-8 of s; max_index -> argmin anchor idx
            DMA:  indirect gather of fused row [V+init+b2 | 2*Key] (f32)
  step k+1  PE:   transpose gathered 2*Key rows -> [4, tok] (PSUM)
            GPS:  rlts = xh[0:4] - ket  (= [2x - 2Key[idx]; junk]^T)
  step k+2  PE:   h^T = (0.5*w1)^T @ rlts (K=4)
            ACT:  ht = gelu(h^T + b1) -> bf16
  step k+4  PE:   o = ht^T @ w2(bf16) accumulated over 8 K-chunks, then
                  += V-row via identity matmul; DMA PSUM -> DRAM.

Host: packs layouts, runs 8 cores via run_bass_kernel_spmd, re-assembles,
and re-resolves near-tie tokens (top-2 gap below tau) with exact fp32
reference arithmetic so fp32r matmul rounding cannot flip the argmin.
"""

import numpy as np

B, N, A, D_IN, D_OUT = 8, 8192, 1024, 3, 256
H = 4 * D_OUT
P = 128
NT = N // P            # 64 token tiles per core
TPS = 4                # token tiles per pipeline step
NSTEP = NT // TPS      # 16 steps
SC = TPS * P           # 512 tokens per step
VT_W = 272             # gather-table row width (256 V + 3 key + pad), 1088B
HC = H // P            # 8 H-chunks
TR_LAG = 2             # transpose+subtract phase lag (steps)
H_LAG = 3              # h-MLP phase lag (steps)
OG_LAG = 5             # out-GEMM phase lag (steps)
N_CORES = 8

_PROGRAM = None  # (nc, input_names)


def _build_program():
    import concourse.bass as bass
    import concourse.mybir as mybir
    import concourse.tile as tile
    from concourse import bacc

    f32 = mybir.dt.float32
    f32r = mybir.dt.float32r
    bf16 = mybir.dt.bfloat16
    u32 = mybir.dt.uint32

    # Bacc (not raw Bass): its compile() splits multi-sem waits and moves
    # matmul waits onto ldweights — TRN2 allows at most 1 wait per instr.
    nc = bacc.Bacc("TRN2", target_bir_lowering=False, debug=False)

    # DRAM I/O (xh/kh hold feature rows 0-4 duplicated at 5-9 so they can
    # be DMA'd to SBUF partitions 0-4 and 32-36 for row-group packing).
    xh_d = nc.dram_tensor("xh", [10, N], f32r, kind="ExternalInput").ap()
    kh_d = nc.dram_tensor("kh", [10, A], f32r, kind="ExternalInput").ap()
    w1h_d = nc.dram_tensor("w1h", [4, H], f32r, kind="ExternalInput").ap()
    b1p_d = nc.dram_tensor("b1p", [P, HC], f32, kind="ExternalInput").ap()
    w2p_d = nc.dram_tensor("w2p", [P, HC * D_OUT], bf16, kind="ExternalInput").ap()
    vt_d = nc.dram_tensor("vt", [A, VT_W], f32, kind="ExternalInput").ap()
    id_d = nc.dram_tensor("ident", [P, P], f32r, kind="ExternalInput").ap()

    out_d = nc.dram_tensor("outp", [N, D_OUT], f32, kind="ExternalOutput").ap()
    m8_d = nc.dram_tensor("m8o", [P, NT * 8], f32, kind="ExternalOutput").ap()
    idx_d = nc.dram_tensor("idxo", [P, NT * 8], u32, kind="ExternalOutput").ap()

    with tile.TileContext(nc) as tc:
        with (
            tc.tile_pool(name="const", bufs=1) as cpool,
            tc.tile_pool(name="xh", bufs=6) as xhpool,
            tc.tile_pool(name="vg", bufs=28) as vgpool,
            tc.tile_pool(name="rlts", bufs=4) as rltspool,
            tc.tile_pool(name="ob", bufs=4) as obpool,
            tc.tile_pool(name="ht", bufs=40) as htpool,
            tc.tile_pool(name="m8", bufs=1) as m8pool,
            tc.tile_pool(name="idx", bufs=6) as idxpool,
            tc.tile_pool(name="ps", bufs=2, space="PSUM") as pspool,
        ):
            # Resident constants. kh first (needed by the very first s-mm).
            kh_t = cpool.tile([P, A], f32r)
            nc.sync.dma_start(out=kh_t[0:5, :], in_=kh_d[0:5, :])
            nc.sync.dma_start(out=kh_t[32:37, :], in_=kh_d[5:10, :])
            w1h_t = cpool.tile([4, H], f32r)
            nc.sync.dma_start(out=w1h_t[:], in_=w1h_d[:])
            b1p_t = cpool.tile([P, HC], f32)
            nc.sync.dma_start(out=b1p_t[:], in_=b1p_d[:])
            id_t = cpool.tile([P, P], f32r)
            nc.sync.dma_start(out=id_t[:], in_=id_d[:])
            w2p_t = cpool.tile([P, HC * D_OUT], bf16)
            nc.sync.dma_start(out=w2p_t[:], in_=w2p_d[:])
            m8_t = m8pool.tile([P, NT * 8], f32)

            xh_ts = {}

            def fetch_xh(k):
                xh_t = xhpool.tile([P, SC], f32r, tag="xh")
                nc.sync.dma_start(out=xh_t[0:5, :], in_=xh_d[0:5, k * SC : (k + 1) * SC])
                nc.sync.dma_start(out=xh_t[32:37, :], in_=xh_d[5:10, k * SC : (k + 1) * SC])
                xh_ts[k] = xh_t

            fetch_xh(0)
            fetch_xh(1)

            vgs = {}      # k -> [vg tile per tile-in-step]
            idxs = {}     # k -> idx tile [P, TPS, 8]
            rlts_k = {}   # k -> rlts tile [4, SC]
            hts = {}      # k -> [ht tile per hc]

            def emit_s_pair(k, j):
                """Score matmuls + DVE scan for tile j of step k."""
                t = k * TPS + j
                s_ps = pspool.tile([P, A], f32, tag="s")
                for g in range(2):
                    nc.tensor.matmul(
                        out=s_ps[:, g * 512 : (g + 1) * 512],
                        lhsT=xh_ts[k][32 * g : 32 * g + 5, j * P : (j + 1) * P],
                        rhs=kh_t[32 * g : 32 * g + 5, g * 512 : (g + 1) * 512],
                        start=True,
                        stop=True,
                        tile_position=(32 * g, 0),
                    )
                m8 = m8_t[:, t * 8 : (t + 1) * 8]
                nc.vector.max(m8, s_ps[:])
                nc.vector.max_index(idxs[k][:, j, :], m8, s_ps[:])

            def emit_gather(k, j):
                # NB: one offset per partition ([P,1]) per call into an
                # offset-0 [P, W] dest tile — both multi-index offsets and
                # non-zero dest offsets are mishandled by the real SWDGE
                # (CoreSim accepts them but hardware does not).
                vg_j = vgpool.tile([P, VT_W], f32r, tag="vg")
                nc.gpsimd.indirect_dma_start(
                    out=vg_j[:],
                    out_offset=None,
                    in_=vt_d[:],
                    in_offset=bass.IndirectOffsetOnAxis(ap=idxs[k][:, j, 0:1], axis=0),
                )
                vgs[k].append(vg_j)

            def emit_og(k, j):
                """Out-GEMM for tile j of step k: 8 bf16 K-chunks + V row."""
                t = k * TPS + j
                o_ps = pspool.tile([P, D_OUT], f32, tag="o")
                for hc in range(HC):
                    nc.tensor.matmul(
                        out=o_ps[:],
                        lhsT=hts[k][hc][:, j * P : (j + 1) * P],
                        rhs=w2p_t[:, hc * D_OUT : (hc + 1) * D_OUT],
                        start=(hc == 0),
                        stop=(hc == HC - 1),
                    )
                ob = obpool.tile([P, D_OUT], f32, tag="ob", name="ob")
                nc.vector.tensor_tensor(
                    out=ob[:],
                    in0=o_ps[:],
                    in1=vgs[k][j][:, 0:D_OUT],
                    op=mybir.AluOpType.add,
                )
                nc.sync.dma_start(out=out_d[t * P : (t + 1) * P, :], in_=ob[:])

            def emit_h_one(k, hc):
                """One h-chunk matmul + gelu for step k."""
                if True:
                    h_ps = pspool.tile([P, SC], f32, tag="h", name="hps")
                    nc.tensor.matmul(
                        out=h_ps[:],
                        lhsT=w1h_t[:, hc * P : (hc + 1) * P],
                        rhs=rlts_k[k][:],
                        start=True,
                        stop=True,
                    )
                    ht = htpool.tile([P, SC], bf16, tag="ht", name="ht")
                    nc.scalar.activation(
                        out=ht[:],
                        in_=h_ps[:],
                        func=mybir.ActivationFunctionType.Gelu,
                        bias=b1p_t[:, hc : hc + 1],
                    )
                    hts[k].append(ht)

            for k in range(NSTEP + OG_LAG):
                a = k < NSTEP                          # score/scan/gather
                trp = TR_LAG <= k < NSTEP + TR_LAG     # transpose+subtract
                hp = H_LAG <= k < NSTEP + H_LAG        # h-MLP
                og = OG_LAG <= k                       # out-GEMM

                if a:
                    if k + 2 < NSTEP:
                        fetch_xh(k + 2)
                    idxs[k] = idxpool.tile([P, TPS, 8], u32, tag="idx", name="idxt")
                    vgs[k] = []
                if hp:
                    hts[k - H_LAG] = []

                # PE order per slot j: s-pair(k) | out-GEMM group(k-5) |
                # 2 h-chunks(k-3), with transposes(k-2)+subtract after
                # slot 1.  The deep lags give every cross-engine dep >1
                # step of slack; the slot interleave keeps each PSUM ring
                # (s bufs=2, h bufs=2, o bufs=2) comfortably ahead of its
                # consumers so the PE stream never stalls (any bubble
                # drops the PE clock from 2.4GHz to 1.2GHz for ~3us).
                for j in range(TPS):
                    if a:
                        emit_s_pair(k, j)
                    if hp:
                        emit_h_one(k - H_LAG, 2 * j)
                    if og:
                        emit_og(k - OG_LAG, j)
                    if hp:
                        emit_h_one(k - H_LAG, 2 * j + 1)
                    if j == 1 and trp:
                        kk = k - TR_LAG
                        rlt_ps = pspool.tile([4, SC], f32r, tag="s", name="rltps")
                        for jj in range(TPS):
                            nc.tensor.transpose(
                                out=rlt_ps[:, jj * P : (jj + 1) * P],
                                in_=vgs[kk][jj][:, D_OUT : D_OUT + 4],
                                identity=id_t[:],
                            )
                        rl = rltspool.tile([4, SC], f32r, tag="rlts", name="rl")
                        nc.vector.tensor_tensor(
                            out=rl[:],
                            in0=xh_ts[kk][0:4, :],
                            in1=rlt_ps[:],
                            op=mybir.AluOpType.subtract,
                        )
                        rlts_k[kk] = rl

                if a:
                    # gathers follow their scans on GpSimd
                    for j in range(TPS):
                        emit_gather(k, j)
                    nc.sync.dma_start(
                        out=idx_d[:, k * TPS * 8 : (k + 1) * TPS * 8],
                        in_=idxs[k][:],
                    )

            nc.sync.dma_start(out=m8_d[:], in_=m8_t[:])

    nc.compile()
    names = ["xh", "kh", "w1h", "b1p", "w2p", "vt", "ident"]
    return nc, names


def _get_program():
    global _PROGRAM
    if _PROGRAM is None:
        _PROGRAM = _build_program()
    return _PROGRAM


def _host_pack(x, Key, init_mat, Value, w1, b1, w2, b2):
    """Build per-core input dicts (host-side layout packing)."""
    import ml_dtypes

    f = np.float32
    Key = np.asarray(Key, f)
    x = np.asarray(x, f)
    k2 = np.sum(Key * Key, axis=1)  # [A]

    # khat rows: [k0,k1,k2,1,|k|^2]; s = 2x.k - |x|^2 - |k|^2 = -d2
    kf = np.concatenate([Key, np.ones((A, 1), f), k2[:, None]], axis=1)  # [A,5]
    kh = np.concatenate([kf.T, kf.T], axis=0)  # [10, A]

    w1h = np.zeros((4, H), f)
    w1h[:3, :] = 0.5 * np.asarray(w1, f)
    b1p = np.asarray(b1, f).reshape(HC, P).T.copy()  # [128, 8]
    w2p = (
        np.asarray(w2, f)
        .reshape(HC, P, D_OUT)
        .transpose(1, 0, 2)
        .reshape(P, HC * D_OUT)
        .astype(ml_dtypes.bfloat16)
    )
    vt = np.zeros((A, VT_W), f)
    vt[:, :D_OUT] = np.asarray(init_mat, f) + np.asarray(Value, f) + np.asarray(b2, f)
    vt[:, D_OUT : D_OUT + 3] = 2.0 * Key
    ident = np.eye(P, dtype=f)

    in_maps = []
    for c in range(N_CORES):
        xc = x[c]  # [N, 3]
        x2sq = np.sum(xc * xc, axis=1)  # [N]
        # xhat features [N, 5]: [2x, -|x|^2, -1]
        xf = np.concatenate(
            [2.0 * xc, -x2sq[:, None], -np.ones((N, 1), f)], axis=1
        ).astype(f)
        # [5, N] with tile t at cols t*128.. (tile-major token order)
        xf_t = xf.reshape(NT, P, 5).transpose(2, 0, 1).reshape(5, N)
        xh = np.concatenate([xf_t, xf_t], axis=0).copy()  # [10, N]

        in_maps.append(
            {
                "xh": xh,
                "kh": kh,
                "w1h": w1h,
                "b1p": b1p,
                "w2p": w2p,
                "vt": vt,
                "ident": ident,
            }
        )
    return in_maps


def _erf(z):
    # Abramowitz-Stegun is not enough; use the exact erf from scipy if
    # present, else jax (available wherever the bass stack runs).
    try:
        from scipy.special import erf

        return erf(z)
    except ImportError:
        import jax

        with jax.default_device(jax.devices("cpu")[0]):
            return np.asarray(jax.scipy.special.erf(np.asarray(z, np.float32)))


def _refine(out, m8o, idxo, x, Key, init_mat, Value, w1, b1, w2, b2, tau=0.03):
    """Re-resolve tokens whose top-2 score gap is within tau (near-ties):
    recompute their argmin + output row in exact fp32 reference arithmetic."""
    f = np.float32
    Key = np.asarray(Key, f)
    V = np.asarray(init_mat, f) + np.asarray(Value, f)
    k2 = np.sum(Key * Key, axis=1)
    n_fixed = 0
    for c in range(out.shape[0]):
        m8 = m8o[c]  # [128, NT*8]
        m0 = m8[:, 0::8]  # [128, NT]
        m1 = m8[:, 1::8]
        gap = m0 - m1  # s-space gap == d2 second - d2 min
        dev_idx = idxo[c][:, 0::8].astype(np.int64)  # [128, NT]
        scale = 1.0 + np.abs(m0)
        flag = gap < tau * scale  # [128, NT]
        ps, ts = np.nonzero(flag)
        if ps.size == 0:
            continue
        toks = ts * P + ps
        xc = np.asarray(x[c], f)[toks]  # [F, 3]
        d2 = -2.0 * (xc @ Key.T) + k2[None, :]  # reference formula, fp32
        amin = np.argmin(d2, axis=1)
        mism = amin != dev_idx[ps, ts]
        if not np.any(mism):
            continue
        toks = toks[mism]
        amin = amin[mism]
        xe = np.asarray(x[c], f)[toks]
        rl = xe - Key[amin]
        pre = (rl @ np.asarray(w1, f) + np.asarray(b1, f)).astype(f)
        h = (0.5 * pre * (1.0 + _erf(pre / np.sqrt(f(2.0))))).astype(f)
        row = (h @ np.asarray(w2, f) + np.asarray(b2, f) + V[amin]).astype(f)
        out[c, toks, :] = row
        n_fixed += toks.size
    return n_fixed


def kernel(**inputs):
    from concourse.bass_utils import run_bass_kernel_spmd

    nc, names = _get_program()
    in_maps = _host_pack(**inputs)
    res = run_bass_kernel_spmd(nc, in_maps, core_ids=list(range(N_CORES)))

    out = np.zeros((B, N, D_OUT), np.float32)
    m8o = np.zeros((B, P, NT * 8), np.float32)
    idxo = np.zeros((B, P, NT * 8), np.uint32)
    for c in range(N_CORES):
        r = res.results[c]
        out[c] = r["outp"]
        m8o[c] = r["m8o"]
        idxo[c] = r["idxo"]

    _refine(out, m8o, idxo, **inputs)
    return out


if __name__ == "__main__":
    # smoke: build only
    _get_program()
    print("program built")


# revision 11
# speedup vs baseline: 1.0879x; 1.0201x over previous
"""FAISS-anchor kernel layer on 8 Trainium2 NeuronCores (Bass/Tile).

Problem (per full input):
    x [8,8192,3], Key [1024,3], init_mat/Value [1024,256],
    w1 [3,1024], b1 [1024], w2 [1024,256], b2 [256]
    idx = argmin_a ||x - Key_a||^2           (exact 1-NN, first-tie)
    out = gelu((x - Key[idx]) @ w1 + b1) @ w2 + b2 + (init_mat + Value)[idx]

Sharding: pure data-parallel — core c takes batch element c (8192 tokens).
All tables (Key-derived features, V-table, MLP weights) are replicated.

Device pipeline, software-pipelined in 16 half-chunk steps of 4 token
tiles (512 tokens) so the PE instruction stream stays gapless (TRN2 PE
DVFS only reaches 2.4 GHz after ~3us of continuous execution):
  step k    PE:   s = -||x-k||^2 for 1024 anchors (2 row-group-packed
                  f32r matmuls per tile, interleaved with out-GEMMs of
                  step k-4)
            DVE:  max8 -> top-8 of s; max_index -> argmin anchor idx
            DMA:  indirect gather of fused row [V+init+b2 | 2*Key] (f32)
  step k+1  PE:   transpose gathered 2*Key rows -> [4, tok] (PSUM)
            GPS:  rlts = xh[0:4] - ket  (= [2x - 2Key[idx]; junk]^T)
  step k+2  PE:   h^T = (0.5*w1)^T @ rlts (K=4)
            ACT:  ht = gelu(h^T + b1) -> bf16
  step k+4  PE:   o = ht^T @ w2(bf16) accumulated over 8 K-chunks, then
                  += V-row via identity matmul; DMA PSUM -> DRAM.

Host: packs layouts, runs 8 cores via run_bass_kernel_spmd, re-assembles,
and re-resolves near-tie tokens (top-2 gap below tau) with exact fp32
reference arithmetic so fp32r matmul rounding cannot flip the argmin.
"""

import numpy as np

B, N, A, D_IN, D_OUT = 8, 8192, 1024, 3, 256
H = 4 * D_OUT
P = 128
NT = N // P            # 64 token tiles per core
TPS = 4                # token tiles per pipeline step
NSTEP = NT // TPS      # 16 steps
SC = TPS * P           # 512 tokens per step
VT_W = 272             # gather-table row width (256 V + 3 key + pad), 1088B
HC = H // P            # 8 H-chunks
TR_LAG = 2             # transpose+subtract phase lag (steps)
H_LAG = 3              # h-MLP phase lag (steps)
OG_LAG = 5             # out-GEMM phase lag (steps)
N_CORES = 8

_PROGRAM = None  # (nc, input_names)


def _build_program():
    import concourse.bass as bass
    import concourse.mybir as mybir
    import concourse.tile as tile
    from concourse import bacc

    f32 = mybir.dt.float32
    f32r = mybir.dt.float32r
    bf16 = mybir.dt.bfloat16
    u32 = mybir.dt.uint32

    # Bacc (not raw Bass): its compile() splits multi-sem waits and moves
    # matmul waits onto ldweights — TRN2 allows at most 1 wait per instr.
    nc = bacc.Bacc("TRN2", target_bir_lowering=False, debug=False)

    # DRAM I/O (xh/kh hold feature rows 0-4 duplicated at 5-9 so they can
    # be DMA'd to SBUF partitions 0-4 and 32-36 for row-group packing).
    xh_d = nc.dram_tensor("xh", [10, N], f32r, kind="ExternalInput").ap()
    kh_d = nc.dram_tensor("kh", [10, A], f32r, kind="ExternalInput").ap()
    w1h_d = nc.dram_tensor("w1h", [4, H], f32r, kind="ExternalInput").ap()
    b1p_d = nc.dram_tensor("b1p", [P, HC], f32, kind="ExternalInput").ap()
    w2p_d = nc.dram_tensor("w2p", [P, HC * D_OUT], bf16, kind="ExternalInput").ap()
    vt_d = nc.dram_tensor("vt", [A, VT_W], f32, kind="ExternalInput").ap()
    id_d = nc.dram_tensor("ident", [P, P], f32r, kind="ExternalInput").ap()

    out_d = nc.dram_tensor("outp", [N, D_OUT], f32, kind="ExternalOutput").ap()
    m8_d = nc.dram_tensor("m8o", [P, NT * 8], f32, kind="ExternalOutput").ap()
    idx_d = nc.dram_tensor("idxo", [P, NT * 8], u32, kind="ExternalOutput").ap()

    with tile.TileContext(nc) as tc:
        with (
            tc.tile_pool(name="const", bufs=1) as cpool,
            tc.tile_pool(name="xh", bufs=6) as xhpool,
            tc.tile_pool(name="vg", bufs=28) as vgpool,
            tc.tile_pool(name="rlts", bufs=4) as rltspool,
            tc.tile_pool(name="ob", bufs=4) as obpool,
            tc.tile_pool(name="ht", bufs=40) as htpool,
            tc.tile_pool(name="m8", bufs=1) as m8pool,
            tc.tile_pool(name="idx", bufs=6) as idxpool,
            tc.tile_pool(name="ps", bufs=2, space="PSUM") as pspool,
        ):
            # Resident constants; kh + the first two xh chunks are issued
            # first so the score pipeline starts immediately — the bulky
            # MLP tables (w2p) are only needed ~OG_LAG steps in.
            kh_t = cpool.tile([P, A], f32r)
            nc.sync.dma_start(out=kh_t[0:5, :], in_=kh_d[0:5, :])
            nc.sync.dma_start(out=kh_t[32:37, :], in_=kh_d[5:10, :])

            xh_ts = {}

            def fetch_xh(k):
                xh_t = xhpool.tile([P, SC], f32r, tag="xh")
                nc.sync.dma_start(out=xh_t[0:5, :], in_=xh_d[0:5, k * SC : (k + 1) * SC])
                nc.sync.dma_start(out=xh_t[32:37, :], in_=xh_d[5:10, k * SC : (k + 1) * SC])
                xh_ts[k] = xh_t

            fetch_xh(0)
            fetch_xh(1)

            w1h_t = cpool.tile([4, H], f32r)
            nc.sync.dma_start(out=w1h_t[:], in_=w1h_d[:])
            b1p_t = cpool.tile([P, HC], f32)
            nc.sync.dma_start(out=b1p_t[:], in_=b1p_d[:])
            id_t = cpool.tile([P, P], f32r)
            nc.sync.dma_start(out=id_t[:], in_=id_d[:])
            w2p_t = cpool.tile([P, HC * D_OUT], bf16)
            nc.sync.dma_start(out=w2p_t[:], in_=w2p_d[:])
            m8_t = m8pool.tile([P, NT * 8], f32)

            vgs = {}      # k -> [vg tile per tile-in-step]
            idxs = {}     # k -> idx tile [P, TPS, 8]
            rlts_k = {}   # k -> rlts tile [4, SC]
            hts = {}      # k -> [ht tile per hc]

            def emit_s_pair(k, j):
                """Score matmuls + DVE scan for tile j of step k."""
                t = k * TPS + j
                s_ps = pspool.tile([P, A], f32, tag="s")
                for g in range(2):
                    nc.tensor.matmul(
                        out=s_ps[:, g * 512 : (g + 1) * 512],
                        lhsT=xh_ts[k][32 * g : 32 * g + 5, j * P : (j + 1) * P],
                        rhs=kh_t[32 * g : 32 * g + 5, g * 512 : (g + 1) * 512],
                        start=True,
                        stop=True,
                        tile_position=(32 * g, 0),
                    )
                m8 = m8_t[:, t * 8 : (t + 1) * 8]
                nc.vector.max(m8, s_ps[:])
                nc.vector.max_index(idxs[k][:, j, :], m8, s_ps[:])

            def emit_gather(k, j):
                # NB: one offset per partition ([P,1]) per call into an
                # offset-0 [P, W] dest tile — both multi-index offsets and
                # non-zero dest offsets are mishandled by the real SWDGE
                # (CoreSim accepts them but hardware does not).
                vg_j = vgpool.tile([P, VT_W], f32r, tag="vg")
                nc.gpsimd.indirect_dma_start(
                    out=vg_j[:],
                    out_offset=None,
                    in_=vt_d[:],
                    in_offset=bass.IndirectOffsetOnAxis(ap=idxs[k][:, j, 0:1], axis=0),
                )
                vgs[k].append(vg_j)

            def emit_og(k, j):
                """Out-GEMM for tile j of step k: 8 bf16 K-chunks + V row."""
                t = k * TPS + j
                o_ps = pspool.tile([P, D_OUT], f32, tag="o")
                for hc in range(HC):
                    nc.tensor.matmul(
                        out=o_ps[:],
                        lhsT=hts[k][hc][:, j * P : (j + 1) * P],
                        rhs=w2p_t[:, hc * D_OUT : (hc + 1) * D_OUT],
                        start=(hc == 0),
                        stop=(hc == HC - 1),
                    )
                ob = obpool.tile([P, D_OUT], f32, tag="ob", name="ob")
                nc.vector.tensor_tensor(
                    out=ob[:],
                    in0=o_ps[:],
                    in1=vgs[k][j][:, 0:D_OUT],
                    op=mybir.AluOpType.add,
                )
                nc.sync.dma_start(out=out_d[t * P : (t + 1) * P, :], in_=ob[:])

            def emit_h_one(k, hc):
                """One h-chunk matmul + gelu for step k."""
                if True:
                    h_ps = pspool.tile([P, SC], f32, tag="h", name="hps")
                    nc.tensor.matmul(
                        out=h_ps[:],
                        lhsT=w1h_t[:, hc * P : (hc + 1) * P],
                        rhs=rlts_k[k][:],
                        start=True,
                        stop=True,
                    )
                    ht = htpool.tile([P, SC], bf16, tag="ht", name="ht")
                    nc.scalar.activation(
                        out=ht[:],
                        in_=h_ps[:],
                        func=mybir.ActivationFunctionType.Gelu,
                        bias=b1p_t[:, hc : hc + 1],
                    )
                    hts[k].append(ht)

            for k in range(NSTEP + OG_LAG):
                a = k < NSTEP                          # score/scan/gather
                trp = TR_LAG <= k < NSTEP + TR_LAG     # transpose+subtract
                hp = H_LAG <= k < NSTEP + H_LAG        # h-MLP
                og = OG_LAG <= k                       # out-GEMM

                if a:
                    if k + 2 < NSTEP:
                        fetch_xh(k + 2)
                    idxs[k] = idxpool.tile([P, TPS, 8], u32, tag="idx", name="idxt")
                    vgs[k] = []
                if hp:
                    hts[k - H_LAG] = []

                # PE order per slot j: s-pair(k) | out-GEMM group(k-5) |
                # 2 h-chunks(k-3), with transposes(k-2)+subtract after
                # slot 1.  The deep lags give every cross-engine dep >1
                # step of slack; the slot interleave keeps each PSUM ring
                # (s bufs=2, h bufs=2, o bufs=2) comfortably ahead of its
                # consumers so the PE stream never stalls (any bubble
                # drops the PE clock from 2.4GHz to 1.2GHz for ~3us).
                for j in range(TPS):
                    if a:
                        emit_s_pair(k, j)
                    if hp:
                        emit_h_one(k - H_LAG, 2 * j)
                    if og:
                        emit_og(k - OG_LAG, j)
                    if hp:
                        emit_h_one(k - H_LAG, 2 * j + 1)
                    if j == 1 and trp:
                        kk = k - TR_LAG
                        rlt_ps = pspool.tile([4, SC], f32r, tag="s", name="rltps")
                        for jj in range(TPS):
                            nc.tensor.transpose(
                                out=rlt_ps[:, jj * P : (jj + 1) * P],
                                in_=vgs[kk][jj][:, D_OUT : D_OUT + 4],
                                identity=id_t[:],
                            )
                        rl = rltspool.tile([4, SC], f32r, tag="rlts", name="rl")
                        nc.vector.tensor_tensor(
                            out=rl[:],
                            in0=xh_ts[kk][0:4, :],
                            in1=rlt_ps[:],
                            op=mybir.AluOpType.subtract,
                        )
                        rlts_k[kk] = rl

                if a:
                    # gathers follow their scans on GpSimd
                    for j in range(TPS):
                        emit_gather(k, j)
                    nc.sync.dma_start(
                        out=idx_d[:, k * TPS * 8 : (k + 1) * TPS * 8],
                        in_=idxs[k][:],
                    )

            nc.sync.dma_start(out=m8_d[:], in_=m8_t[:])

    nc.compile()
    names = ["xh", "kh", "w1h", "b1p", "w2p", "vt", "ident"]
    return nc, names


def _get_program():
    global _PROGRAM
    if _PROGRAM is None:
        _PROGRAM = _build_program()
    return _PROGRAM


def _host_pack(x, Key, init_mat, Value, w1, b1, w2, b2):
    """Build per-core input dicts (host-side layout packing)."""
    import ml_dtypes

    f = np.float32
    Key = np.asarray(Key, f)
    x = np.asarray(x, f)
    k2 = np.sum(Key * Key, axis=1)  # [A]

    # khat rows: [k0,k1,k2,1,|k|^2]; s = 2x.k - |x|^2 - |k|^2 = -d2
    kf = np.concatenate([Key, np.ones((A, 1), f), k2[:, None]], axis=1)  # [A,5]
    kh = np.concatenate([kf.T, kf.T], axis=0)  # [10, A]

    w1h = np.zeros((4, H), f)
    w1h[:3, :] = 0.5 * np.asarray(w1, f)
    b1p = np.asarray(b1, f).reshape(HC, P).T.copy()  # [128, 8]
    w2p = (
        np.asarray(w2, f)
        .reshape(HC, P, D_OUT)
        .transpose(1, 0, 2)
        .reshape(P, HC * D_OUT)
        .astype(ml_dtypes.bfloat16)
    )
    vt = np.zeros((A, VT_W), f)
    vt[:, :D_OUT] = np.asarray(init_mat, f) + np.asarray(Value, f) + np.asarray(b2, f)
    vt[:, D_OUT : D_OUT + 3] = 2.0 * Key
    ident = np.eye(P, dtype=f)

    in_maps = []
    for c in range(N_CORES):
        xc = x[c]  # [N, 3]
        x2sq = np.sum(xc * xc, axis=1)  # [N]
        # xhat features [N, 5]: [2x, -|x|^2, -1]
        xf = np.concatenate(
            [2.0 * xc, -x2sq[:, None], -np.ones((N, 1), f)], axis=1
        ).astype(f)
        # [5, N] with tile t at cols t*128.. (tile-major token order)
        xf_t = xf.reshape(NT, P, 5).transpose(2, 0, 1).reshape(5, N)
        xh = np.concatenate([xf_t, xf_t], axis=0).copy()  # [10, N]

        in_maps.append(
            {
                "xh": xh,
                "kh": kh,
                "w1h": w1h,
                "b1p": b1p,
                "w2p": w2p,
                "vt": vt,
                "ident": ident,
            }
        )
    return in_maps


def _erf(z):
    # Abramowitz-Stegun is not enough; use the exact erf from scipy if
    # present, else jax (available wherever the bass stack runs).
    try:
        from scipy.special import erf

        return erf(z)
    except ImportError:
        import jax

        with jax.default_device(jax.devices("cpu")[0]):
            return np.asarray(jax.scipy.special.erf(np.asarray(z, np.float32)))


def _refine(out, m8o, idxo, x, Key, init_mat, Value, w1, b1, w2, b2, tau=0.03):
    """Re-resolve tokens whose top-2 score gap is within tau (near-ties):
    recompute their argmin + output row in exact fp32 reference arithmetic."""
    f = np.float32
    Key = np.asarray(Key, f)
    V = np.asarray(init_mat, f) + np.asarray(Value, f)
    k2 = np.sum(Key * Key, axis=1)
    n_fixed = 0
    for c in range(out.shape[0]):
        m8 = m8o[c]  # [128, NT*8]
        m0 = m8[:, 0::8]  # [128, NT]
        m1 = m8[:, 1::8]
        gap = m0 - m1  # s-space gap == d2 second - d2 min
        dev_idx = idxo[c][:, 0::8].astype(np.int64)  # [128, NT]
        scale = 1.0 + np.abs(m0)
        flag = gap < tau * scale  # [128, NT]
        ps, ts = np.nonzero(flag)
        if ps.size == 0:
            continue
        toks = ts * P + ps
        xc = np.asarray(x[c], f)[toks]  # [F, 3]
        d2 = -2.0 * (xc @ Key.T) + k2[None, :]  # reference formula, fp32
        amin = np.argmin(d2, axis=1)
        mism = amin != dev_idx[ps, ts]
        if not np.any(mism):
            continue
        toks = toks[mism]
        amin = amin[mism]
        xe = np.asarray(x[c], f)[toks]
        rl = xe - Key[amin]
        pre = (rl @ np.asarray(w1, f) + np.asarray(b1, f)).astype(f)
        h = (0.5 * pre * (1.0 + _erf(pre / np.sqrt(f(2.0))))).astype(f)
        row = (h @ np.asarray(w2, f) + np.asarray(b2, f) + V[amin]).astype(f)
        out[c, toks, :] = row
        n_fixed += toks.size
    return n_fixed


def kernel(**inputs):
    from concourse.bass_utils import run_bass_kernel_spmd

    nc, names = _get_program()
    in_maps = _host_pack(**inputs)
    res = run_bass_kernel_spmd(nc, in_maps, core_ids=list(range(N_CORES)))

    out = np.zeros((B, N, D_OUT), np.float32)
    m8o = np.zeros((B, P, NT * 8), np.float32)
    idxo = np.zeros((B, P, NT * 8), np.uint32)
    for c in range(N_CORES):
        r = res.results[c]
        out[c] = r["outp"]
        m8o[c] = r["m8o"]
        idxo[c] = r["idxo"]

    _refine(out, m8o, idxo, **inputs)
    return out


if __name__ == "__main__":
    # smoke: build only
    _get_program()
    print("program built")
